# revision 1
# baseline (speedup 1.0000x reference)
"""GAT 2-layer kernel for 8 Trainium2 NeuronCores.

Strategy (dst-node sharded, gather-based):
  - Nodes sharded 6250/core (padded to 6272 = 49*128). Edges (incl self
    loops) grouped by dst into per-core lists, sorted by dst-group of 128
    ("group"), padded to blocks of 128 edges.
  - Phase A: h = x @ W1_ext per shard (W1_ext cols: 96 h | asrc | adst | 1)
    -> fp16 table shard; AllGather -> full table [50176, 99].
  - Edge phase per group g (128 dst nodes, nb blocks of 128 edges):
      * one indirect DMA per block gathers 128 table rows by src id
      * S = exp(leakyrelu(M01 + adst_col + asrc) - 4) built via batched
        DVE/ACT ops; M01 is a host-shipped fp8 mask (0 at edge's dst col,
        -448 elsewhere); adst via PE transpose-broadcast; asrc rides in
        gathered col 96.
      * matmul accumulate PSUM[128 dst, 99] += S_b.T @ H_b over blocks;
        col 98 of table is 1.0 so PSUM col 98 = softmax denominator.
      * out1 = ELU(PSUM[:, :96]/denom + b1)
  - Layer2 feature: h2 = out1 @ W2_ext (transpose via PE), AllGather,
    same edge phase with R=35, then log_softmax -> out shard fp32.
"""
import sys
import time

sys.path.insert(0, "/opt/trn_rl_repo")

import numpy as np

N = 50000
E = 800000
F_IN = 256
HID = 96
N_CLS = 32
NEG_SLOPE = 0.2
N_CORES = 8
P = 128
NSH = 6250            # real nodes per core
NSHP = 6272           # padded (49 * 128)
NG = NSHP // P        # 49 groups per core
R1 = HID + 3          # table1 row: h(96) | asrc | adst | one  = 99
R2 = N_CLS + 3        # table2 row: h2(32) | asrc2 | adst2 | one = 35
TAB = N_CORES * NSHP  # 50176
MASKVAL = -448.0
EXP_BIAS = -4.0

_CACHE = {}


def _host_prep(edge_index):
    """Pure index preprocessing: per-core padded edge blocks + masks."""
    src = np.asarray(edge_index[0], dtype=np.int64)
    dst = np.asarray(edge_index[1], dtype=np.int64)
    loops = np.arange(N, dtype=np.int64)
    src = np.concatenate([src, loops])
    dst = np.concatenate([dst, loops])
    order = np.argsort(dst, kind="stable")
    src = src[order]
    dst = dst[order]
    # global table row id for a node (core-local padded layout)
    tab_row = (src // NSH) * NSHP + (src % NSH)

    cores = []
    for c in range(N_CORES):
        lo, hi = c * NSH, (c + 1) * NSH
        a, b = np.searchsorted(dst, [lo, hi])
        s_c = tab_row[a:b]
        d_c = dst[a:b] - lo  # 0..6249 local
        g_c = d_c // P
        # per-group edge lists padded to multiples of 128
        idx_blocks = []   # per block: [128] int32 table rows
        rel_blocks = []   # per block: [128] int16 dst col (0..127), -1 pad
        gofs = [0]
        nb_per_g = []
        for g in range(NG):
            m = g_c == g
            sg = s_c[m].astype(np.int64)
            rg = (d_c[m] - g * P).astype(np.int64)
            if g == NG - 1 and len(sg) == 0:
                pass
            # ensure every node of the group has >=1 edge (real data always
            # does via self-loops except pad nodes; pad nodes get one pad
            # edge pointing at col so denom > 0)
            npad_nodes = []
            present = np.zeros(P, dtype=bool)
            present[rg] = True
            missing = np.nonzero(~present)[0]
            if len(missing):
                sg = np.concatenate([sg, np.zeros(len(missing), np.int64)])
                rg = np.concatenate([rg, missing])
            n = len(sg)
            nb = max(1, -(-n // P))
            pad = nb * P - n
            if pad:
                sg = np.concatenate([sg, np.zeros(pad, np.int64)])
                rg = np.concatenate([rg, np.full(pad, -1, np.int64)])
            idx_blocks.append(sg.reshape(nb, P).astype(np.int32))
            rel_blocks.append(rg.reshape(nb, P).astype(np.int16))
            nb_per_g.append(nb)
            gofs.append(gofs[-1] + nb)
        nbt = gofs[-1]
        idx_all = np.concatenate(idx_blocks, axis=0)          # [nbt, 128]
        rel_all = np.concatenate(rel_blocks, axis=0)          # [nbt, 128]
        # one-hot masks fp8: m01[e, b*128+c] = 1 iff edge(b,e) has dst col c
        # (pad edges rel=-1 -> all-zero row); m10 = per-block transpose.
        cols = np.arange(P, dtype=np.int16)
        oh = (rel_all[:, :, None] == cols[None, None, :]).astype(np.float32)
        m01 = np.transpose(oh, (1, 0, 2)).reshape(P, nbt * P)
        m10 = np.transpose(oh, (2, 0, 1)).reshape(P, nbt * P)
        cores.append(
            dict(
                idx=np.ascontiguousarray(idx_all.T),  # [128, nbt] int32
                m01=m01,
                m10=m10,
                nb_per_g=np.array(nb_per_g, np.int32),
                nbt=nbt,
            )
        )
    return cores


def _split_multi_waits(nc):
    """TRN2 ISA has one sync-wait slot per instruction; Tile sometimes emits
    more. Hoist extras onto preceding single-wait Drain pseudo-instructions."""
    import concourse.mybir as mybir

    for f in nc.m.functions:
        for blk in f.blocks:
            out = []
            for inst in blk.instructions:
                si = inst.sync_info
                if si is not None and len(si.on_wait) > 1:
                    waits = list(si.on_wait)
                    for w in waits[:-1]:
                        d = mybir.InstNoOp(
                            name=nc.get_next_instruction_name(),
                            ins=[], outs=[],
                        )
                        d.engine = inst.engine
                        d.sync_info = mybir.SyncInfo(on_wait=[w], on_update=[])
                        out.append(d)
                    inst.sync_info = mybir.SyncInfo(
                        on_wait=[waits[-1]], on_update=list(si.on_update))
                out.append(inst)
            blk.instructions = out


class _SpmdRunner:
    """Persistent jitted 8-core runner (mirrors bass2jax.run_bass_via_pjrt)."""

    def __init__(self, nc, n_cores=8):
        import jax
        from jax.sharding import Mesh, PartitionSpec
        from jax.experimental.shard_map import shard_map
        import concourse.mybir as mybir
        from concourse.bass2jax import (
            _bass_exec_p, install_neuronx_cc_hook, partition_id_tensor)

        install_neuronx_cc_hook()
        self.jax = jax
        self.n_cores = n_cores
        pname = nc.partition_id_tensor.name if nc.partition_id_tensor else None
        in_names, out_names, out_avals, zero_outs = [], [], [], []
        for alloc in nc.m.functions[0].allocations:
            if not isinstance(alloc, mybir.MemoryLocationSet):
                continue
            name = alloc.memorylocations[0].name
            if alloc.kind == "ExternalInput":
                if name != pname:
                    in_names.append(name)
            elif alloc.kind == "ExternalOutput":
                shape = tuple(alloc.tensor_shape)
                dtype = mybir.dt.np(alloc.dtype)
                out_names.append(name)
                out_avals.append(jax.core.ShapedArray(shape, dtype))
                zero_outs.append(np.zeros(shape, dtype))
        self.in_names, self.out_names = in_names, out_names
        self.out_avals, self.zero_outs = out_avals, zero_outs
        n_params, n_outs = len(in_names), len(out_avals)
        all_in = in_names + out_names + ([pname] if pname else [])

        def _body(*args):
            operands = list(args)
            if pname is not None:
                operands.append(partition_id_tensor())
            return tuple(_bass_exec_p.bind(
                *operands, out_avals=tuple(out_avals), in_names=tuple(all_in),
                out_names=tuple(out_names), lowering_input_output_aliases=(),
                sim_require_finite=True, sim_require_nnan=True, nc=nc))

        devices = jax.devices()[:n_cores]
        mesh = Mesh(np.asarray(devices), ("core",))
        self.fn = jax.jit(
            shard_map(_body, mesh=mesh,
                      in_specs=(PartitionSpec("core"),) * (n_params + n_outs),
                      out_specs=(PartitionSpec("core"),) * len(out_names),
                      check_rep=False),
            donate_argnums=tuple(range(n_params, n_params + n_outs)),
            keep_unused=True)
        self.n_params = n_params
        self.staged = None

    def stage(self, in_maps):
        jax = self.jax
        per_core = [[np.asarray(m[n]) for n in self.in_names] for m in in_maps]
        self.staged = jax.device_put([
            np.concatenate([per_core[c][i] for c in range(self.n_cores)], axis=0)
            for i in range(self.n_params)])
        jax.block_until_ready(self.staged)

    def _zeros(self):
        return [np.zeros((self.n_cores * z.shape[0], *z.shape[1:]), z.dtype)
                for z in self.zero_outs]

    def run_results(self):
        jax = self.jax
        out = self.fn(*self.staged, *self._zeros())
        jax.block_until_ready(out)
        return [
            {n: np.asarray(out[i]).reshape(self.n_cores, *self.out_avals[i].shape)[c]
             for i, n in enumerate(self.out_names)}
            for c in range(self.n_cores)]

    def time_min(self, iters=6, warmup=2):
        jax = self.jax
        for _ in range(warmup):
            jax.block_until_ready(self.fn(*self.staged, *self._zeros()))
        ts = []
        for _ in range(iters):
            z = jax.device_put(self._zeros())
            jax.block_until_ready(z)
            t0 = time.perf_counter()
            jax.block_until_ready(self.fn(*self.staged, *z))
            ts.append(time.perf_counter() - t0)
        return min(ts)


def _build_nc(nb_per_g, nbt, ablate=()):
    import concourse.bass as bass
    import concourse.mybir as mybir
    import concourse.tile as tile

    fp16 = mybir.dt.float16
    fp32 = mybir.dt.float32
    fp8 = mybir.dt.float8e4
    AO = mybir.AluOpType
    AF = mybir.ActivationFunctionType

    nc = bass.Bass()
    xT = nc.declare_dram_parameter("xT", [F_IN, NSHP], fp32, isOutput=False)
    w1 = nc.declare_dram_parameter("w1", [F_IN, R1], fp32, isOutput=False)
    w2 = nc.declare_dram_parameter("w2", [HID, R2], fp32, isOutput=False)
    b1r = nc.declare_dram_parameter("b1r", [1, HID], fp32, isOutput=False)
    b2r = nc.declare_dram_parameter("b2r", [1, N_CLS], fp32, isOutput=False)
    iden = nc.declare_dram_parameter("iden", [P, P], fp16, isOutput=False)
    idxT = nc.declare_dram_parameter("idxT", [P, nbt], mybir.dt.int32, isOutput=False)
    m01 = nc.declare_dram_parameter("m01", [P, nbt * P], mybir.dt.float8e4, isOutput=False)
    m10 = nc.declare_dram_parameter("m10", [P, nbt * P], mybir.dt.float8e4, isOutput=False)
    out = nc.declare_dram_parameter("out", [NSH, N_CLS], fp32, isOutput=True)

    with tile.TileContext(nc) as tc:
        with (
            tc.tile_pool(name="const", bufs=1) as cp,
            tc.tile_pool(name="sb", bufs=3) as sb,
            tc.tile_pool(name="ps", bufs=3, space="PSUM") as ps,
            tc.tile_pool(name="pt", bufs=2, space="PSUM") as pt,
            tc.tile_pool(name="dram", bufs=1, space="DRAM") as dr,
        ):
            ident = cp.tile([P, P], fp16)
            nc.sync.dma_start(out=ident[:], in_=iden[:])
            w1t = cp.tile([P, 2, R1], fp16)
            nc.gpsimd.dma_start(out=w1t[:], in_=w1[:].rearrange("(k p) r -> p k r", p=P))
            w2t = cp.tile([HID, R2], fp16)
            nc.gpsimd.dma_start(out=w2t[:], in_=w2[:])
            idx_sb = cp.tile([P, nbt], mybir.dt.int32)
            nc.sync.dma_start(out=idx_sb[:], in_=idxT[:])

            # b replicated tiles via transpose trick: load b as [1, F] then
            # matmul ones[1,128].T @ b -> [128, F]
            ones1 = cp.tile([1, P], fp16)
            nc.vector.memset(ones1[:], 1.0)
            b1h = cp.tile([1, HID], fp16)
            nc.gpsimd.dma_start(out=b1h[:], in_=b1r[:])
            b2h = cp.tile([1, N_CLS], fp16)
            nc.gpsimd.dma_start(out=b2h[:], in_=b2r[:])
            b1ps = pt.tile([P, HID], fp32, space="PSUM", tag="tp")
            nc.tensor.matmul(out=b1ps[:], lhsT=ones1[:], rhs=b1h[:], start=True, stop=True)
            b1rep = cp.tile([P, HID], fp32)
            nc.vector.tensor_copy(b1rep[:], b1ps[:])
            b2ps = pt.tile([P, N_CLS], fp32, space="PSUM", tag="tp")
            nc.tensor.matmul(out=b2ps[:], lhsT=ones1[:], rhs=b2h[:], start=True, stop=True)
            b2rep = cp.tile([P, N_CLS], fp32)
            nc.vector.tensor_copy(b2rep[:], b2ps[:])
            neg4 = cp.tile([P, 1], fp32)
            nc.vector.memset(neg4[:], EXP_BIAS)

            tab1_sh = dr.tile([NSHP, R1], fp16)
            tab1_cc = dr.tile([TAB, R1], fp16, addr_space="Shared")
            tab1 = dr.tile([TAB, R1], fp16)
            tab2_sh = dr.tile([NSHP, R2], fp16)
            tab2_cc = dr.tile([TAB, R2], fp16, addr_space="Shared")
            tab2 = dr.tile([TAB, R2], fp16)

            # ---- Phase A: h table shard ----
            xTc = cp.tile([P, 2, NSHP], fp16)
            nc.gpsimd.dma_start(out=xTc[:], in_=xT[:].rearrange("(k p) n -> p k n", p=P))
            for g in range(NG):
                hps = ps.tile([P, R1], fp32, space="PSUM", tag="agg")
                for k in range(2):
                    nc.tensor.matmul(
                        out=hps[:], lhsT=xTc[:, k, g * P:(g + 1) * P],
                        rhs=w1t[:, k, :], start=(k == 0), stop=(k == 1),
                    )
                hsb = sb.tile([P, R1], fp16, tag="hsb")
                nc.scalar.activation(out=hsb[:, 0:R1 - 1], in_=hps[:, 0:R1 - 1],
                                     func=AF.Copy, bias=0.0, scale=1.0)
                nc.vector.memset(hsb[:, R1 - 1:R1], 1.0)
                nc.sync.dma_start(out=tab1_sh[g * P:(g + 1) * P, :], in_=hsb[:])
            if "nocollective" in ablate:
                nc.sync.dma_start(out=tab1_cc[0:NSHP, :], in_=tab1_sh[:])
            else:
                nc.gpsimd.collective_compute(
                    "AllGather", mybir.AluOpType.bypass,
                    replica_groups=[list(range(N_CORES))],
                    ins=[tab1_sh.opt()], outs=[tab1_cc.opt()],
                )
            nc.sync.dma_start(out=tab1[:], in_=tab1_cc[:])

            # adst is read from the *local shard* table (tab1_sh/tab2_sh):
            # a group's 128 dst nodes are core-local rows, so no partition
            # id is needed.
            def run_layer(table_sh, table, RL, FL, brep, tag, post):
                boff = 0
                for g in range(NG):
                    nb = int(nb_per_g[g])
                    gt = sb.tile([P, nb, RL], fp16, tag=f"gt{tag}", bufs=6)
                    if "nogather" in ablate:
                        nc.sync.dma_start(
                            out=gt[:], in_=table[0:P * nb, 0:RL].rearrange(
                                "(b p) r -> p b r", p=P))
                    else:
                        for b in range(nb):
                            nc.gpsimd.indirect_dma_start(
                                out=gt[:, b, :], out_offset=None, in_=table[:],
                                in_offset=bass.IndirectOffsetOnAxis(
                                    ap=idx_sb[:, boff + b:boff + b + 1], axis=0),
                            )
                    adst = sb.tile([P, 1], fp16, tag=f"ad{tag}")
                    nc.sync.dma_start(
                        out=adst[:],
                        in_=table_sh[g * P:(g + 1) * P, RL - 2:RL - 1],
                    )
                    m01s = sb.tile([P, nb, P], fp8, tag=f"m01{tag}")
                    nc.sync.dma_start(
                        out=m01s[:],
                        in_=m01[:, boff * P:(boff + nb) * P].rearrange(
                            "p (b c) -> p b c", c=P),
                    )
                    m10s = sb.tile([P, nb, P], fp8, tag=f"m10{tag}")
                    nc.sync.dma_start(
                        out=m10s[:],
                        in_=m10[:, boff * P:(boff + nb) * P].rearrange(
                            "p (b c) -> p b c", c=P),
                    )
                    # per-edge dst logit: pre[e, b] = adst[dst(e)] via one-hot
                    # transposed mask matmul per block
                    pre = ps.tile([P, nb, 1], fp32, space="PSUM", tag="pre")
                    for b in range(nb):
                        nc.tensor.matmul(
                            out=pre[:, b, :], lhsT=m10s[:, b, :], rhs=adst[:],
                            start=True, stop=True,
                        )
                    # w = exp(leakyrelu(pre + asrc) - 4)  [P, nb, 1]
                    z = sb.tile([P, nb, 1], fp32, tag=f"z{tag}")
                    nc.vector.tensor_tensor(
                        out=z[:], in0=pre[:], in1=gt[:, :, RL - 3:RL - 2],
                        op=AO.add,
                    )
                    e02 = sb.tile([P, nb, 1], fp32, tag=f"e02{tag}")
                    nc.vector.tensor_scalar(
                        out=e02[:], in0=z[:], scalar1=NEG_SLOPE,
                        scalar2=None, op0=AO.mult,
                    )
                    nc.vector.tensor_tensor(out=z[:], in0=z[:], in1=e02[:],
                                            op=AO.max)
                    w = sb.tile([P, nb, 1], fp16, tag=f"w{tag}")
                    nc.scalar.activation(out=w[:], in_=z[:],
                                         func=AF.Exp, bias=neg4[:], scale=1.0)
                    # weighted rows: gt2 = [w*h | w]
                    gt2 = sb.tile([P, nb, FL + 1], fp16, tag=f"gt2{tag}")
                    nc.vector.tensor_tensor(
                        out=gt2[:, :, 0:FL], in0=gt[:, :, 0:FL],
                        in1=w[:].to_broadcast([P, nb, FL]), op=AO.mult,
                    )
                    nc.vector.tensor_copy(gt2[:, :, FL:FL + 1], w[:])
                    # aggregate: agg[d, :] = sum_e onehot[e,d] * gt2[e, :]
                    agg = ps.tile([P, FL + 1], fp32, space="PSUM", tag="agg")
                    for b in range(nb):
                        nc.tensor.matmul(
                            out=agg[:], lhsT=m01s[:, b, :], rhs=gt2[:, b, :],
                            start=(b == 0), stop=(b == nb - 1),
                        )
                    # divide + bias
                    rcp = sb.tile([P, 1], fp32, tag=f"rcp{tag}")
                    nc.vector.reciprocal(rcp[:], agg[:, FL:FL + 1])
                    o = sb.tile([P, FL], fp32, tag=f"o{tag}")
                    nc.vector.tensor_scalar(
                        out=o[:], in0=agg[:, 0:FL], scalar1=rcp[:],
                        scalar2=None, op0=AO.mult,
                    )
                    nc.vector.tensor_tensor(out=o[:], in0=o[:], in1=brep[:],
                                            op=AO.add)
                    post(g, o)
                    boff += nb

            def post1(g, o):
                # elu = max(u,0) + exp(min(u,0)) - 1
                mn = sb.tile([P, HID], fp32, tag="mn")
                nc.vector.tensor_scalar(out=mn[:], in0=o[:], scalar1=0.0,
                                        scalar2=None, op0=AO.min)
                ex = sb.tile([P, HID], fp32, tag="ex")
                nc.scalar.activation(out=ex[:], in_=mn[:], func=AF.Exp,
                                     bias=0.0, scale=1.0)
                mx = sb.tile([P, HID], fp16, tag="mx")
                nc.vector.tensor_scalar(out=mx[:], in0=o[:], scalar1=0.0,
                                        scalar2=-1.0, op0=AO.max, op1=AO.add)
                elu = sb.tile([P, HID], fp16, tag="elu")
                nc.vector.tensor_tensor(out=elu[:], in0=ex[:], in1=mx[:],
                                        op=AO.add)
                # transpose [128, 96] -> [96, 128]
                elups = pt.tile([HID, P], fp16, space="PSUM", tag="tp")
                nc.tensor.transpose(out=elups[:], in_=elu[:], identity=ident[:])
                eluT = sb.tile([HID, P], fp16, tag="eluT")
                nc.vector.tensor_copy(eluT[:], elups[:])
                h2ps = ps.tile([P, R2], fp32, space="PSUM", tag="agg")
                nc.tensor.matmul(out=h2ps[:], lhsT=eluT[:], rhs=w2t[:],
                                 start=True, stop=True)
                h2sb = sb.tile([P, R2], fp16, tag="h2sb")
                nc.scalar.activation(out=h2sb[:, 0:R2 - 1], in_=h2ps[:, 0:R2 - 1],
                                     func=AF.Copy, bias=0.0, scale=1.0)
                nc.vector.memset(h2sb[:, R2 - 1:R2], 1.0)
                nc.sync.dma_start(out=tab2_sh[g * P:(g + 1) * P, :], in_=h2sb[:])

            def post2(g, o):
                if g * P >= NSH:
                    return
                mx2 = sb.tile([P, 1], fp32, tag="mx2")
                nc.vector.tensor_reduce(
                    out=mx2[:], in_=o[:], op=AO.max,
                    axis=mybir.AxisListType.X,
                )
                t = sb.tile([P, N_CLS], fp32, tag="t2")
                nc.vector.tensor_scalar(out=t[:], in0=o[:], scalar1=mx2[:],
                                        scalar2=None, op0=AO.subtract)
                exs = sb.tile([P, N_CLS], fp32, tag="exs")
                ssum = sb.tile([P, 1], fp32, tag="ssum")
                nc.scalar.activation(out=exs[:], in_=t[:], func=AF.Exp,
                                     bias=0.0, scale=1.0, accum_out=ssum[:])
                lse = sb.tile([P, 1], fp32, tag="lse")
                nc.scalar.activation(out=lse[:], in_=ssum[:], func=AF.Ln,
                                     bias=0.0, scale=1.0)
                fo = sb.tile([P, N_CLS], fp32, tag="fo")
                nc.vector.tensor_scalar(out=fo[:], in0=t[:], scalar1=lse[:],
                                        scalar2=None, op0=AO.subtract)
                hi = min((g + 1) * P, NSH)
                nc.sync.dma_start(out=out[g * P:hi, :], in_=fo[: hi - g * P, :])

            run_layer(tab1_sh, tab1, R1, HID, b1rep, "L1", post1)
            if "nocollective" in ablate:
                nc.sync.dma_start(out=tab2_cc[0:NSHP, :], in_=tab2_sh[:])
            else:
                nc.gpsimd.collective_compute(
                    "AllGather", mybir.AluOpType.bypass,
                    replica_groups=[list(range(N_CORES))],
                    ins=[tab2_sh.opt()], outs=[tab2_cc.opt()],
                )
            nc.sync.dma_start(out=tab2[:], in_=tab2_cc[:])
            run_layer(tab2_sh, tab2, R2, N_CLS, b2rep, "L2", post2)

    _split_multi_waits(nc)
    return nc


def kernel(x, edge_index, W1, a1_src, a1_dst, b1, W2, a2_src, a2_dst, b2):
    x = np.asarray(x, np.float32)
    key = "prep"
    if key not in _CACHE:
        _CACHE[key] = _host_prep(edge_index)
    cores = _CACHE[key]

    # all cores share nb_per_g? No - per-core nbt differ. Build per-core nc?
    # SPMD requires ONE program: pad all cores to the same nbt/nb_per_g.
    nbmax = np.max([np.array([c["nb_per_g"] for c in cores])], axis=0)
    # unify: per-group nb = max over cores
    nb_uni = np.max(np.stack([c["nb_per_g"] for c in cores]), axis=0)
    nbt_uni = int(nb_uni.sum())
    in_maps = []
    W1e = np.concatenate(
        [W1, (W1 @ a1_src)[:, None], (W1 @ a1_dst)[:, None],
         np.zeros((F_IN, 1), np.float32)], axis=1)
    W2e = np.concatenate(
        [W2, (W2 @ a2_src)[:, None], (W2 @ a2_dst)[:, None],
         np.zeros((HID, 1), np.float32)], axis=1)
    iden = np.eye(P, dtype=np.float16)
    for c in range(N_CORES):
        d = cores[c]
        # re-pad this core's blocks to nb_uni (pad blocks: idx 0, masks 0)
        idx = d["idx"]  # [128, nbt_c]
        m01c = d["m01"]
        m10c = d["m10"]
        nbg = d["nb_per_g"]
        idx_new = np.zeros((P, nbt_uni), np.int32)
        m01_new = np.zeros((P, nbt_uni * P), np.float32)
        m10_new = np.zeros((P, nbt_uni * P), np.float32)
        src_off = 0
        dst_off = 0
        for g in range(NG):
            nb_c = int(nbg[g])
            nb_u = int(nb_uni[g])
            idx_new[:, dst_off:dst_off + nb_c] = idx[:, src_off:src_off + nb_c]
            m01_new[:, dst_off * P:(dst_off + nb_c) * P] = \
                m01c[:, src_off * P:(src_off + nb_c) * P]
            m10_new[:, dst_off * P:(dst_off + nb_c) * P] = \
                m10c[:, src_off * P:(src_off + nb_c) * P]
            src_off += nb_c
            dst_off += nb_u
        xs = x[c * NSH:(c + 1) * NSH]
        xT = np.zeros((F_IN, NSHP), np.float32)
        xT[:, :NSH] = xs.T
        in_maps.append({
            "xT": xT, "w1": W1e, "w2": W2e,
            "b1r": np.asarray(b1, np.float32)[None, :],
            "b2r": np.asarray(b2, np.float32)[None, :],
            "iden": iden, "idxT": idx_new,
            "m01": _to_fp8(m01_new),
            "m10": _to_fp8(m10_new),
        })

    if "runner" not in _CACHE:
        nc = _build_nc(nb_uni, nbt_uni)
        _CACHE["runner"] = _SpmdRunner(nc, N_CORES)
    run = _CACHE["runner"]
    run.stage(in_maps)
    res = run.run_results()
    return np.concatenate([res[c]["out"] for c in range(N_CORES)], axis=0)


def measure_hw_ns(iters=6):
    """Steady-state wall time of the staged kernel minus a no-op dispatch
    baseline of the same I/O shape class (axon per-execution overhead)."""
    run = _CACHE.get("runner")
    assert run is not None and run.staged is not None, "call kernel() first"
    if "noop" not in _CACHE:
        import concourse.bass as bass
        import concourse.mybir as mybir
        import concourse.tile as tile
        nc = bass.Bass()
        a = nc.declare_dram_parameter("a", [128, 32], mybir.dt.float32, isOutput=False)
        o = nc.declare_dram_parameter("out", [128, 32], mybir.dt.float32, isOutput=True)
        with tile.TileContext(nc) as tc:
            with tc.tile_pool(name="s", bufs=1) as sb:
                t = sb.tile([128, 32], mybir.dt.float32)
                nc.sync.dma_start(out=t[:], in_=a[:])
                nc.sync.dma_start(out=o[:], in_=t[:])
        _split_multi_waits(nc)
        nr = _SpmdRunner(nc, N_CORES)
        nr.stage([{"a": np.zeros((128, 32), np.float32)}] * N_CORES)
        _CACHE["noop"] = nr
    nr = _CACHE["noop"]
    # interleave to cancel slow drift in axon dispatch overhead
    reals, bases = [], []
    for _ in range(max(iters, 8)):
        bases.append(nr.time_min(iters=1, warmup=0))
        reals.append(run.time_min(iters=1, warmup=0))
    return max(int((min(reals) - min(bases)) * 1e9), 1000)


def _to_fp8(a):
    import ml_dtypes
    return a.astype(ml_dtypes.float8_e4m3)

